# revision 7
# baseline (speedup 1.0000x reference)
"""Trainium2 Bass kernel for BasisFunction1D (piecewise-linear basis / histogram binning).

Math:
  out[o, b] = sum_i (1-d)*P[g, o, i] + d*P[g+1, o, i],
  g = bucket of x[i,b] on the borders grid, d = in-bucket linear position.

As a function of x, each per-i contribution is continuous piecewise-linear
with knots at borders[1..127] plus linear extrapolation at both ends.  Such a
function decomposes into a sum of window clamps:

  out = Cconst + sum_c Dq_c^T @ clamp(x - beta_m, a_lo_c, a_hi_c)

Extrapolation is absorbed by opening chunk 0's lower bound and chunk 127's
upper bound to +-1e4 (the window basis then extends linearly with the edge
slopes, exactly the reference's extrapolation semantics).

Device algorithm (per core, batch shard of 1024):
  1. ACT: t_m = fp16(x - beta_m) for 32 groups of 4 chunks (group centering
     keeps values small so fp16 is accurate and matmul products stay tiny).
  2. DVE: R_c = clamp(t_m, a_lo_c, a_hi_c), one fp16 tensor_scalar (max,min)
     per chunk -- runs in the DVE 4x perf mode (~340ns/pass).
  3. PE: acc += Dq_c^T @ R_c accumulated over 129 fp16 matmul pairs in PSUM
     (128 chunks + constant via a ones matmul).

Numerics: a_lo/a_hi are fp16-grid values, so saturated clamp outputs are
exact.  The lhs tables are fp16-quantized with error feedback across c
("noise shaping") so per-chunk rise errors never accumulate; the constant
anchors the function at the all-saturated-low state.  Host-sim rel err of
this exact scheme vs the reference: ~6.5e-4.

For benching (reps>1) the body is emitted twice inside a For_i over reps/2
with all tiles double-buffered, so consecutive iterations pipeline: the
x-DMA -> ACT -> DVE head and the PSUM-copy -> DMA tail of one iteration
overlap the matmul stream of its neighbours.
"""

import hashlib

import numpy as np

I_DIM = 128
O_DIM = 128
G = 128
B_FULL = 8192
N_CORES = 8
BS = B_FULL // N_CORES

GRP = 8                 # chunks per centering group
M_GRPS = G // GRP       # 32 groups
NBLK = G + 1            # lhs blocks: [Crows, Dq_0..Dq_127]
BIG = 1.0e4             # open bound for the extrapolating edge chunks

_NC_CACHE = {}
_RUNNER_CACHE = {}


def _f16(a):
    return np.asarray(a, dtype=np.float32).astype(np.float16)


def _make_tables(P, borders, inv_len):
    """Build fp16 lhs table [I, NBLK*O] and per-chunk scalar constants."""
    P64 = np.asarray(P, dtype=np.float64)
    b32 = np.asarray(borders, dtype=np.float32)
    b64 = b32.astype(np.float64)
    il64 = np.asarray(inv_len, dtype=np.float64)

    D = P64[1:] - P64[:-1]                                   # [G, O, I]
    rise = D * ((b64[1:] - b64[:-1]) * il64)[:, None, None]  # [G, O, I]

    beta = np.array(
        [(b32[GRP * m] + b32[GRP * m + GRP]) * 0.5 for m in range(M_GRPS)],
        dtype=np.float32,
    )
    grp_of = np.arange(G) // GRP
    a_lo = _f16(b32[:G] - beta[grp_of]).astype(np.float32)       # fp16-grid
    a_hi = _f16(b32[1 : G + 1] - beta[grp_of]).astype(np.float32)
    width = a_hi.astype(np.float64) - a_lo.astype(np.float64)

    # noise-shaped fp16 quantization of the slopes: Dq_c*width_c tracks the
    # exact per-chunk rise with error feedback so drift never accumulates
    Dq = np.zeros((G, O_DIM, I_DIM), dtype=np.float32)
    err = np.zeros((O_DIM, I_DIM), dtype=np.float64)
    for c in range(G):
        q = _f16((rise[c] - err) / width[c]).astype(np.float32)
        Dq[c] = q
        err += q.astype(np.float64) * width[c] - rise[c]

    # constant: G(all clamps saturated low) == F(x=b0) == sum_i P0
    C = P64[0].sum(axis=1) - (
        Dq.astype(np.float64) * a_lo.astype(np.float64)[:, None, None]
    ).sum(axis=(0, 2))                                           # [O]
    # spread C over the 128 contraction rows of a ones-matmul, noise-shaped
    Crows = np.zeros((I_DIM, O_DIM), dtype=np.float32)
    rem = C.copy()
    for i in range(I_DIM):
        r = _f16(rem / (I_DIM - i)).astype(np.float32)
        Crows[i] = r
        rem -= r.astype(np.float64)

    blocks = np.zeros((I_DIM, NBLK, O_DIM), dtype=np.float16)
    blocks[:, 0, :] = Crows
    blocks[:, 1:, :] = np.ascontiguousarray(Dq.transpose(2, 0, 1))
    lhs = np.ascontiguousarray(blocks.reshape(I_DIM, NBLK * O_DIM))

    # device clamp bounds: absorb extrapolation into the edge chunks
    dev_lo = a_lo.copy()
    dev_hi = a_hi.copy()
    dev_lo[0] = -BIG
    dev_hi[G - 1] = BIG
    return lhs, beta, dev_lo, dev_hi


def _build_nc(beta, a_lo, a_hi, reps: int = 1):
    from contextlib import ExitStack

    import concourse.bacc as bacc
    import concourse.mybir as mybir
    import concourse.tile as tile

    dt = mybir.dt
    f32 = dt.float32
    f16 = dt.float16
    AF = mybir.ActivationFunctionType
    OP = mybir.AluOpType

    nc = bacc.Bacc("TRN2", target_bir_lowering=False, debug=False)

    x_d = nc.dram_tensor("x", [I_DIM, BS], f32, kind="ExternalInput")
    lhs_d = nc.dram_tensor("lhs", [I_DIM, NBLK * O_DIM], f16, kind="ExternalInput")
    out_d = nc.dram_tensor("out", [O_DIM, BS], f32, kind="ExternalOutput")

    HALF = BS // 2
    if reps > 1:
        assert reps % 2 == 0
        unroll, iters = 2, reps // 2
    else:
        unroll, iters = 1, 1

    with tile.TileContext(nc) as tc, ExitStack() as ctx:
        if iters > 1:
            loop_cm = tc.For_i(
                0,
                iters,
                1,
                hint_engines=(
                    mybir.EngineType.PE,
                    mybir.EngineType.Activation,
                    mybir.EngineType.DVE,
                ),
            )
            ctx.enter_context(loop_cm)
        dbuf = ctx.enter_context(tc.tile_pool(name="dbuf", bufs=2))
        tpool = ctx.enter_context(tc.tile_pool(name="tpool", bufs=6))
        rpool = ctx.enter_context(tc.tile_pool(name="rpool", bufs=12))
        opsum = ctx.enter_context(tc.tile_pool(name="opsum", bufs=2, space="PSUM"))

        bounds = [0, 16, 72, NBLK]  # lhs DMA chunk boundaries

        def body():
            x_sb = dbuf.tile([I_DIM, BS], f32, tag="x", name="x")
            nc.sync.dma_start(x_sb[:], x_d.ap())

            lhs_tiles = []  # (tile, first_block)
            for j in range(len(bounds) - 1):
                lo, hi = bounds[j], bounds[j + 1]
                ct = dbuf.tile(
                    [I_DIM, (hi - lo) * O_DIM], f16, tag=f"lhs{j}", name=f"lhs{j}"
                )
                nc.sync.dma_start(ct[:], lhs_d.ap()[:, lo * O_DIM : hi * O_DIM])
                lhs_tiles.append((ct, lo))

            def lhsT(blk):
                for ct, lo in reversed(lhs_tiles):
                    if blk >= lo:
                        return ct[:, (blk - lo) * O_DIM : (blk - lo + 1) * O_DIM]
                raise AssertionError(blk)

            ones = dbuf.tile([I_DIM, BS], f16, tag="ones", name="ones")
            nc.vector.memset(ones[:], 1.0)

            acc = opsum.tile([O_DIM, BS], f32, tag="acc", name="acc")

            def mm(blk, rhs, start=False, stop=False):
                w = lhsT(blk)
                nc.tensor.matmul(
                    acc[:, 0:HALF], w, rhs[:, 0:HALF],
                    start=start, stop=stop, skip_group_check=True,
                )
                nc.tensor.matmul(
                    acc[:, HALF:BS], w, rhs[:, HALF:BS],
                    start=start, stop=stop, skip_group_check=True,
                )

            mm(0, ones, start=True)  # constant rows @ ones

            def make_t(m):
                # gpsimd is ~10x too slow for these and ACT alone measures
                # ~4.5us/pass on HW, so alternate DVE and ACT: each then
                # produces one t per ~6.8us of PE stream, which both sustain
                t = tpool.tile([I_DIM, BS], f16, tag="t", name="t")
                if m % 2 == 0:
                    nc.vector.tensor_scalar(
                        t[:], x_sb[:], float(beta[m]), None, OP.subtract
                    )
                else:
                    nc.scalar.activation(
                        t[:], x_sb[:], AF.Copy, bias=float(-beta[m])
                    )
                return t

            # t production runs 2 groups ahead of the R/matmul stream so the
            # DVE always has banked lookahead and jitter never stalls the PE
            t_tiles = {0: make_t(0), 1: make_t(1)}
            for m in range(M_GRPS):
                if m + 2 < M_GRPS:
                    t_tiles[m + 2] = make_t(m + 2)
                t = t_tiles.pop(m)
                for k in range(GRP):
                    c = GRP * m + k
                    r = rpool.tile([I_DIM, BS], f16, tag="r", name="r")
                    nc.vector.tensor_scalar(
                        r[:], t[:], float(a_lo[c]), float(a_hi[c]), OP.max, OP.min
                    )
                    mm(1 + c, r, stop=(c == G - 1))

            out_sb = dbuf.tile([O_DIM, BS], f32, tag="osb", name="osb")
            nc.vector.tensor_copy(out_sb[:, 0:HALF], acc[:, 0:HALF])
            nc.scalar.copy(out_sb[:, HALF:BS], acc[:, HALF:BS])
            nc.sync.dma_start(out_d.ap(), out_sb[:])

        for _ in range(unroll):
            body()

    return nc


def _get_nc(tab_key, tables, reps: int = 1):
    key = (tab_key, reps)
    if key not in _NC_CACHE:
        _, beta, a_lo, a_hi = tables
        _NC_CACHE[key] = _build_nc(beta, a_lo, a_hi, reps)
    return _NC_CACHE[key]


def _host_inputs(x, func_parameter, borders, inverse_chunk_lengths):
    x = np.ascontiguousarray(np.asarray(x, dtype=np.float32))
    P = np.asarray(func_parameter, dtype=np.float32)
    borders = np.asarray(borders, dtype=np.float32)
    inv_len = np.asarray(inverse_chunk_lengths, dtype=np.float32)

    tab_key = hashlib.sha1(borders.tobytes() + inv_len.tobytes()).hexdigest()
    tables = _make_tables(P, borders, inv_len)
    lhs = tables[0]

    in_maps = []
    for c in range(N_CORES):
        xs = np.ascontiguousarray(x[:, c * BS : (c + 1) * BS])
        in_maps.append({"x": xs, "lhs": lhs})
    return in_maps, (tab_key, tables)


def _get_runner(ctx, reps: int = 1):
    """Cached jitted 8-core runner (mirrors bass2jax.run_bass_via_pjrt multi-core path)."""
    tab_key, tables = ctx
    key = (tab_key, reps)
    if key in _RUNNER_CACHE:
        return _RUNNER_CACHE[key]

    import jax
    from jax.sharding import Mesh, PartitionSpec
    from jax.experimental.shard_map import shard_map
    import concourse.mybir as mybir
    from concourse.bass2jax import (
        _bass_exec_p,
        install_neuronx_cc_hook,
        partition_id_tensor,
    )

    install_neuronx_cc_hook()
    nc = _get_nc(tab_key, tables, reps)
    if not nc.is_finalized():
        nc.finalize()
    assert nc.dbg_addr is None
    partition_name = nc.partition_id_tensor.name if nc.partition_id_tensor else None

    in_names, out_names, out_avals, zero_outs = [], [], [], []
    for alloc in nc.m.functions[0].allocations:
        if not isinstance(alloc, mybir.MemoryLocationSet):
            continue
        name = alloc.memorylocations[0].name
        if alloc.kind == "ExternalInput":
            if name != partition_name:
                in_names.append(name)
        elif alloc.kind == "ExternalOutput":
            shape = tuple(alloc.tensor_shape)
            dtype = mybir.dt.np(alloc.dtype)
            out_names.append(name)
            out_avals.append(jax.core.ShapedArray(shape, dtype))
            zero_outs.append(np.zeros(shape, dtype))
    n_params = len(in_names)
    all_names = in_names + out_names
    if partition_name is not None:
        all_names = all_names + [partition_name]

    def _body(*args):
        operands = list(args)
        if partition_name is not None:
            operands.append(partition_id_tensor())
        outs = _bass_exec_p.bind(
            *operands,
            out_avals=tuple(out_avals),
            in_names=tuple(all_names),
            out_names=tuple(out_names),
            lowering_input_output_aliases=(),
            sim_require_finite=True,
            sim_require_nnan=True,
            nc=nc,
        )
        return tuple(outs)

    devices = jax.devices()[:N_CORES]
    mesh = Mesh(np.asarray(devices), ("core",))
    n_outs = len(out_names)
    sharded = jax.jit(
        shard_map(
            _body,
            mesh=mesh,
            in_specs=(PartitionSpec("core"),) * (n_params + n_outs),
            out_specs=(PartitionSpec("core"),) * n_outs,
            check_rep=False,
        ),
        keep_unused=True,
    )

    def run(in_maps):
        concat_in = [
            np.concatenate([np.asarray(m[name]) for m in in_maps], axis=0)
            for name in in_names
        ]
        concat_zero = [
            np.zeros((N_CORES * z.shape[0], *z.shape[1:]), z.dtype) for z in zero_outs
        ]
        out_arrs = sharded(*concat_in, *concat_zero)
        res = [
            {
                name: np.asarray(out_arrs[i]).reshape(N_CORES, *out_avals[i].shape)[c]
                for i, name in enumerate(out_names)
            }
            for c in range(N_CORES)
        ]
        return res, (sharded, concat_in, concat_zero)

    _RUNNER_CACHE[key] = run
    return run


def _run(in_maps, ctx):
    run = _get_runner(ctx)
    results, _ = run(in_maps)
    out = np.concatenate([r["out"] for r in results], axis=1)
    return np.ascontiguousarray(out.astype(np.float32)), results


def bench(in_maps, ctx, iters=30, reps=1):
    """Return (best_per_exec_seconds, times list) by timing repeated dispatches."""
    import time
    import jax

    run = _get_runner(ctx, reps)
    _, (sharded, concat_in, concat_zero) = run(in_maps)
    din = [jax.device_put(a) for a in concat_in]
    dzero = [jax.device_put(a) for a in concat_zero]
    jax.block_until_ready(sharded(*din, *dzero))
    times = []
    for _ in range(iters):
        t0 = time.perf_counter()
        jax.block_until_ready(sharded(*din, *dzero))
        times.append(time.perf_counter() - t0)
    return min(times), times


def bench_device(in_maps, ctx, reps=256, iters=10):
    """Estimate true per-kernel device time: (T_reps - T_1) / (reps - 1),
    cancelling the (dominant) axon dispatch overhead."""
    t1, _ = bench(in_maps, ctx, iters=iters, reps=1)
    tr, _ = bench(in_maps, ctx, iters=iters, reps=reps)
    return (tr - t1) / (reps - 1), t1, tr


def kernel(x, func_parameter, borders, inverse_chunk_lengths):
    in_maps, ctx = _host_inputs(x, func_parameter, borders, inverse_chunk_lengths)
    out, _ = _run(in_maps, ctx)
    return out


def kernel_with_stats(x, func_parameter, borders, inverse_chunk_lengths, trace=True):
    """Returns (out, (in_maps, ctx)) - test harness helper."""
    in_maps, ctx = _host_inputs(x, func_parameter, borders, inverse_chunk_lengths)
    out, results = _run(in_maps, ctx)
    return out, (in_maps, ctx)


# revision 9
# speedup vs baseline: 1.0714x; 1.0714x over previous
"""Trainium2 Bass kernel for BasisFunction1D (piecewise-linear basis / histogram binning).

Math:
  out[o, b] = sum_i (1-d)*P[g, o, i] + d*P[g+1, o, i],
  g = bucket of x[i,b] on the borders grid, d = in-bucket linear position.

As a function of x, each per-i contribution is continuous piecewise-linear
with knots at borders[1..127] plus linear extrapolation at both ends.  Such a
function decomposes into a sum of window clamps:

  out = Cconst + sum_c Dq_c^T @ clamp(x - beta_m, a_lo_c, a_hi_c)

Extrapolation is absorbed by opening chunk 0's lower bound and chunk 127's
upper bound to +-1e4 (the window basis then extends linearly with the edge
slopes, exactly the reference's extrapolation semantics).

Device algorithm (per core, batch shard of 1024):
  1. ACT: t_m = fp16(x - beta_m) for 32 groups of 4 chunks (group centering
     keeps values small so fp16 is accurate and matmul products stay tiny).
  2. DVE: R_c = clamp(t_m, a_lo_c, a_hi_c), one fp16 tensor_scalar (max,min)
     per chunk -- runs in the DVE 4x perf mode (~340ns/pass).
  3. PE: acc += Dq_c^T @ R_c accumulated over 129 fp16 matmul pairs in PSUM
     (128 chunks + constant via a ones matmul).

Numerics: a_lo/a_hi are fp16-grid values, so saturated clamp outputs are
exact.  The lhs tables are fp16-quantized with error feedback across c
("noise shaping") so per-chunk rise errors never accumulate; the constant
anchors the function at the all-saturated-low state.  Host-sim rel err of
this exact scheme vs the reference: ~6.5e-4.

For benching (reps>1) the body is emitted twice inside a For_i over reps/2
with all tiles double-buffered, so consecutive iterations pipeline: the
x-DMA -> ACT -> DVE head and the PSUM-copy -> DMA tail of one iteration
overlap the matmul stream of its neighbours.
"""

import hashlib

import numpy as np

I_DIM = 128
O_DIM = 128
G = 128
B_FULL = 8192
N_CORES = 8
BS = B_FULL // N_CORES

GRP = 16                # chunks per centering group
M_GRPS = G // GRP       # 32 groups
NBLK = G + 1            # lhs blocks: [Crows, Dq_0..Dq_127]
BIG = 1.0e4             # open bound for the extrapolating edge chunks

_NC_CACHE = {}
_RUNNER_CACHE = {}


def _f16(a):
    return np.asarray(a, dtype=np.float32).astype(np.float16)


def _make_tables(P, borders, inv_len):
    """Build fp16 lhs table [I, NBLK*O] and per-chunk scalar constants."""
    P64 = np.asarray(P, dtype=np.float64)
    b32 = np.asarray(borders, dtype=np.float32)
    b64 = b32.astype(np.float64)
    il64 = np.asarray(inv_len, dtype=np.float64)

    D = P64[1:] - P64[:-1]                                   # [G, O, I]
    rise = D * ((b64[1:] - b64[:-1]) * il64)[:, None, None]  # [G, O, I]

    beta = np.array(
        [(b32[GRP * m] + b32[GRP * m + GRP]) * 0.5 for m in range(M_GRPS)],
        dtype=np.float32,
    )
    grp_of = np.arange(G) // GRP
    a_lo = _f16(b32[:G] - beta[grp_of]).astype(np.float32)       # fp16-grid
    a_hi = _f16(b32[1 : G + 1] - beta[grp_of]).astype(np.float32)
    width = a_hi.astype(np.float64) - a_lo.astype(np.float64)

    # noise-shaped fp16 quantization of the slopes: Dq_c*width_c tracks the
    # exact per-chunk rise with error feedback so drift never accumulates
    Dq = np.zeros((G, O_DIM, I_DIM), dtype=np.float32)
    err = np.zeros((O_DIM, I_DIM), dtype=np.float64)
    for c in range(G):
        q = _f16((rise[c] - err) / width[c]).astype(np.float32)
        Dq[c] = q
        err += q.astype(np.float64) * width[c] - rise[c]

    # constant: G(all clamps saturated low) == F(x=b0) == sum_i P0
    C = P64[0].sum(axis=1) - (
        Dq.astype(np.float64) * a_lo.astype(np.float64)[:, None, None]
    ).sum(axis=(0, 2))                                           # [O]
    # spread C over the 128 contraction rows of a ones-matmul, noise-shaped
    Crows = np.zeros((I_DIM, O_DIM), dtype=np.float32)
    rem = C.copy()
    for i in range(I_DIM):
        r = _f16(rem / (I_DIM - i)).astype(np.float32)
        Crows[i] = r
        rem -= r.astype(np.float64)

    blocks = np.zeros((I_DIM, NBLK, O_DIM), dtype=np.float16)
    blocks[:, 0, :] = Crows
    blocks[:, 1:, :] = np.ascontiguousarray(Dq.transpose(2, 0, 1))
    lhs = np.ascontiguousarray(blocks.reshape(I_DIM, NBLK * O_DIM))

    # device clamp bounds: absorb extrapolation into the edge chunks
    dev_lo = a_lo.copy()
    dev_hi = a_hi.copy()
    dev_lo[0] = -BIG
    dev_hi[G - 1] = BIG
    return lhs, beta, dev_lo, dev_hi


def _build_nc(beta, a_lo, a_hi, reps: int = 1):
    from contextlib import ExitStack

    import concourse.bacc as bacc
    import concourse.mybir as mybir
    import concourse.tile as tile

    dt = mybir.dt
    f32 = dt.float32
    f16 = dt.float16
    AF = mybir.ActivationFunctionType
    OP = mybir.AluOpType

    nc = bacc.Bacc("TRN2", target_bir_lowering=False, debug=False)

    x_d = nc.dram_tensor("x", [I_DIM, BS], f32, kind="ExternalInput")
    lhs_d = nc.dram_tensor("lhs", [I_DIM, NBLK * O_DIM], f16, kind="ExternalInput")
    out_d = nc.dram_tensor("out", [O_DIM, BS], f32, kind="ExternalOutput")

    HALF = BS // 2
    if reps > 1:
        assert reps % 2 == 0
        unroll, iters = 2, reps // 2
    else:
        unroll, iters = 1, 1

    with tile.TileContext(nc) as tc, ExitStack() as ctx:
        if iters > 1:
            loop_cm = tc.For_i(
                0,
                iters,
                1,
                hint_engines=(
                    mybir.EngineType.PE,
                    mybir.EngineType.Activation,
                    mybir.EngineType.DVE,
                ),
            )
            ctx.enter_context(loop_cm)
        dbuf = ctx.enter_context(tc.tile_pool(name="dbuf", bufs=2))
        tpool = ctx.enter_context(tc.tile_pool(name="tpool", bufs=6))
        rpool = ctx.enter_context(tc.tile_pool(name="rpool", bufs=12))
        opsum = ctx.enter_context(tc.tile_pool(name="opsum", bufs=2, space="PSUM"))

        bounds = [0, 16, 72, NBLK]  # lhs DMA chunk boundaries

        def body():
            x_sb = dbuf.tile([I_DIM, BS], f32, tag="x", name="x")
            nc.sync.dma_start(x_sb[:], x_d.ap())

            lhs_tiles = []  # (tile, first_block)
            for j in range(len(bounds) - 1):
                lo, hi = bounds[j], bounds[j + 1]
                ct = dbuf.tile(
                    [I_DIM, (hi - lo) * O_DIM], f16, tag=f"lhs{j}", name=f"lhs{j}"
                )
                nc.sync.dma_start(ct[:], lhs_d.ap()[:, lo * O_DIM : hi * O_DIM])
                lhs_tiles.append((ct, lo))

            def lhsT(blk):
                for ct, lo in reversed(lhs_tiles):
                    if blk >= lo:
                        return ct[:, (blk - lo) * O_DIM : (blk - lo + 1) * O_DIM]
                raise AssertionError(blk)

            ones = dbuf.tile([I_DIM, BS], f16, tag="ones", name="ones")
            nc.vector.memset(ones[:], 1.0)

            acc = opsum.tile([O_DIM, BS], f32, tag="acc", name="acc")

            def mm(blk, rhs, start=False, stop=False):
                w = lhsT(blk)
                nc.tensor.matmul(
                    acc[:, 0:HALF], w, rhs[:, 0:HALF],
                    start=start, stop=stop, skip_group_check=True,
                )
                nc.tensor.matmul(
                    acc[:, HALF:BS], w, rhs[:, HALF:BS],
                    start=start, stop=stop, skip_group_check=True,
                )

            mm(0, ones, start=True)  # constant rows @ ones

            def make_t(m):
                # gpsimd and ACT both measure far over their modeled cost on
                # HW (ACT ~4.5us/pass), so t stays on the DVE with the R's
                t = tpool.tile([I_DIM, BS], f16, tag="t", name="t")
                nc.vector.tensor_scalar(
                    t[:], x_sb[:], float(beta[m]), None, OP.subtract
                )
                return t

            # t production runs 2 groups ahead of the R/matmul stream so the
            # DVE always has banked lookahead and jitter never stalls the PE
            t_tiles = {0: make_t(0), 1: make_t(1)}
            for m in range(M_GRPS):
                if m + 2 < M_GRPS:
                    t_tiles[m + 2] = make_t(m + 2)
                t = t_tiles.pop(m)
                for k in range(GRP):
                    c = GRP * m + k
                    r = rpool.tile([I_DIM, BS], f16, tag="r", name="r")
                    nc.vector.tensor_scalar(
                        r[:], t[:], float(a_lo[c]), float(a_hi[c]), OP.max, OP.min
                    )
                    mm(1 + c, r, stop=(c == G - 1))

            out_sb = dbuf.tile([O_DIM, BS], f32, tag="osb", name="osb")
            nc.vector.tensor_copy(out_sb[:, 0:HALF], acc[:, 0:HALF])
            nc.scalar.copy(out_sb[:, HALF:BS], acc[:, HALF:BS])
            nc.sync.dma_start(out_d.ap(), out_sb[:])

        for _ in range(unroll):
            body()

    return nc


def _get_nc(tab_key, tables, reps: int = 1):
    key = (tab_key, reps)
    if key not in _NC_CACHE:
        _, beta, a_lo, a_hi = tables
        _NC_CACHE[key] = _build_nc(beta, a_lo, a_hi, reps)
    return _NC_CACHE[key]


def _host_inputs(x, func_parameter, borders, inverse_chunk_lengths):
    x = np.ascontiguousarray(np.asarray(x, dtype=np.float32))
    P = np.asarray(func_parameter, dtype=np.float32)
    borders = np.asarray(borders, dtype=np.float32)
    inv_len = np.asarray(inverse_chunk_lengths, dtype=np.float32)

    tab_key = hashlib.sha1(borders.tobytes() + inv_len.tobytes()).hexdigest()
    tables = _make_tables(P, borders, inv_len)
    lhs = tables[0]

    in_maps = []
    for c in range(N_CORES):
        xs = np.ascontiguousarray(x[:, c * BS : (c + 1) * BS])
        in_maps.append({"x": xs, "lhs": lhs})
    return in_maps, (tab_key, tables)


def _get_runner(ctx, reps: int = 1):
    """Cached jitted 8-core runner (mirrors bass2jax.run_bass_via_pjrt multi-core path)."""
    tab_key, tables = ctx
    key = (tab_key, reps)
    if key in _RUNNER_CACHE:
        return _RUNNER_CACHE[key]

    import jax
    from jax.sharding import Mesh, PartitionSpec
    from jax.experimental.shard_map import shard_map
    import concourse.mybir as mybir
    from concourse.bass2jax import (
        _bass_exec_p,
        install_neuronx_cc_hook,
        partition_id_tensor,
    )

    install_neuronx_cc_hook()
    nc = _get_nc(tab_key, tables, reps)
    if not nc.is_finalized():
        nc.finalize()
    assert nc.dbg_addr is None
    partition_name = nc.partition_id_tensor.name if nc.partition_id_tensor else None

    in_names, out_names, out_avals, zero_outs = [], [], [], []
    for alloc in nc.m.functions[0].allocations:
        if not isinstance(alloc, mybir.MemoryLocationSet):
            continue
        name = alloc.memorylocations[0].name
        if alloc.kind == "ExternalInput":
            if name != partition_name:
                in_names.append(name)
        elif alloc.kind == "ExternalOutput":
            shape = tuple(alloc.tensor_shape)
            dtype = mybir.dt.np(alloc.dtype)
            out_names.append(name)
            out_avals.append(jax.core.ShapedArray(shape, dtype))
            zero_outs.append(np.zeros(shape, dtype))
    n_params = len(in_names)
    all_names = in_names + out_names
    if partition_name is not None:
        all_names = all_names + [partition_name]

    def _body(*args):
        operands = list(args)
        if partition_name is not None:
            operands.append(partition_id_tensor())
        outs = _bass_exec_p.bind(
            *operands,
            out_avals=tuple(out_avals),
            in_names=tuple(all_names),
            out_names=tuple(out_names),
            lowering_input_output_aliases=(),
            sim_require_finite=True,
            sim_require_nnan=True,
            nc=nc,
        )
        return tuple(outs)

    devices = jax.devices()[:N_CORES]
    mesh = Mesh(np.asarray(devices), ("core",))
    n_outs = len(out_names)
    sharded = jax.jit(
        shard_map(
            _body,
            mesh=mesh,
            in_specs=(PartitionSpec("core"),) * (n_params + n_outs),
            out_specs=(PartitionSpec("core"),) * n_outs,
            check_rep=False,
        ),
        keep_unused=True,
    )

    def run(in_maps):
        concat_in = [
            np.concatenate([np.asarray(m[name]) for m in in_maps], axis=0)
            for name in in_names
        ]
        concat_zero = [
            np.zeros((N_CORES * z.shape[0], *z.shape[1:]), z.dtype) for z in zero_outs
        ]
        out_arrs = sharded(*concat_in, *concat_zero)
        res = [
            {
                name: np.asarray(out_arrs[i]).reshape(N_CORES, *out_avals[i].shape)[c]
                for i, name in enumerate(out_names)
            }
            for c in range(N_CORES)
        ]
        return res, (sharded, concat_in, concat_zero)

    _RUNNER_CACHE[key] = run
    return run


def _run(in_maps, ctx):
    run = _get_runner(ctx)
    results, _ = run(in_maps)
    out = np.concatenate([r["out"] for r in results], axis=1)
    return np.ascontiguousarray(out.astype(np.float32)), results


def bench(in_maps, ctx, iters=30, reps=1):
    """Return (best_per_exec_seconds, times list) by timing repeated dispatches."""
    import time
    import jax

    run = _get_runner(ctx, reps)
    _, (sharded, concat_in, concat_zero) = run(in_maps)
    din = [jax.device_put(a) for a in concat_in]
    dzero = [jax.device_put(a) for a in concat_zero]
    jax.block_until_ready(sharded(*din, *dzero))
    times = []
    for _ in range(iters):
        t0 = time.perf_counter()
        jax.block_until_ready(sharded(*din, *dzero))
        times.append(time.perf_counter() - t0)
    return min(times), times


def bench_device(in_maps, ctx, reps=256, iters=10):
    """Estimate true per-kernel device time: (T_reps - T_1) / (reps - 1),
    cancelling the (dominant) axon dispatch overhead.  The two programs are
    timed in alternation so slow drift of the dispatch overhead (tens of ms
    over minutes) cancels out of the subtraction."""
    import time
    import jax

    runs = []
    for r in (1, reps):
        run = _get_runner(ctx, r)
        _, (sharded, concat_in, concat_zero) = run(in_maps)
        din = [jax.device_put(a) for a in concat_in]
        dzero = [jax.device_put(a) for a in concat_zero]
        jax.block_until_ready(sharded(*din, *dzero))
        runs.append((sharded, din, dzero))
    t1s, trs = [], []
    for _ in range(iters):
        for (sharded, din, dzero), acc in zip(runs, (t1s, trs)):
            t0 = time.perf_counter()
            jax.block_until_ready(sharded(*din, *dzero))
            acc.append(time.perf_counter() - t0)
    t1, tr = min(t1s), min(trs)
    return (tr - t1) / (reps - 1), t1, tr


def kernel(x, func_parameter, borders, inverse_chunk_lengths):
    in_maps, ctx = _host_inputs(x, func_parameter, borders, inverse_chunk_lengths)
    out, _ = _run(in_maps, ctx)
    return out


def kernel_with_stats(x, func_parameter, borders, inverse_chunk_lengths, trace=True):
    """Returns (out, (in_maps, ctx)) - test harness helper."""
    in_maps, ctx = _host_inputs(x, func_parameter, borders, inverse_chunk_lengths)
    out, results = _run(in_maps, ctx)
    return out, (in_maps, ctx)
